# revision 2
# baseline (speedup 1.0000x reference)
"""CARAFE-naive upsampling (N=4, C=256, H=W=64, k=5, g=4, s=2) on 8 TRN2
NeuronCores.

Strategy
--------
Sharding: core c <- (batch n = c//2, group-pair j = c%2). Each core owns 128
feature channels (2 of the 4 mask groups) of one batch image.

Compute: per source row r and w-tile of Wt=16 source columns, the mask
application is a banded matmul on the TensorEngine:

    out[(g,c), cols] += sum_{w'} statT[(g,w'), (g,c)] * B[(g,w'), cols]

statT is the (block-diagonal over the 2 groups) transposed feature row;
B is a *banded* matrix with mask values on shifted diagonals (row w~+dj
pairs source column 16t+w~+dj-2 with output column w~). The 5 row offsets
di accumulate into PSUM over source rows r = h+di-2.

v2 vs the 91us v1 baseline:
- Wt 32 -> 16: the shipped band's inflation is (Wt+4)/5, so bmat drops
  12.4 -> 6.55 MB/core. K drops 72 -> 40 (unpadded; no FWL, but LDWEIGHTS
  (~107ns, scales with the 128 stationary *columns*) hides under the MM
  stream via the PE's 64-deep reorder window).
- h-PAIR matmuls: two adjacent output row-pairs (2H, 2H+1) share the
  (r, t) stationary with di differing by 1, so one matmul with an affine
  hp-step of (hp_stride - di_stride) covers both: N stays 128 and the
  matmul count stays ~6 per (r, t-loop) despite NT doubling.
- PSUM pair tile [c, t(4), hp, w~, a, b] = 2KB = exactly one bank; the
  first MM touching a generation carries start=True (clears the bank's
  has_written bits; later first-touch-of-element MMs with start=False
  overwrite, per-element semantics).
- stat ships dense-40-row (2.62 MB) so there are zero on-device memsets.
"""

import sys

import numpy as np

for _p in ("/opt/trn_rl_repo", "/opt/pypackages"):
    if _p not in sys.path:
        sys.path.append(_p)

import ml_dtypes  # noqa: E402
from contextlib import ExitStack  # noqa: E402

import concourse.bass as bass  # noqa: E402
import concourse.tile as tile  # noqa: E402
from concourse import bacc, mybir  # noqa: E402
from concourse.bass_utils import run_bass_kernel_spmd  # noqa: E402

# Problem constants (hardcoded per harness contract)
KS = 5            # kernel size
S = 2             # upscale
N, C, H, W = 4, 256, 64, 64
Wt = 16           # w-tile
NT = W // Wt      # 4 tiles
KB = Wt + 4       # band rows per group = 20
KK = 2 * KB       # contraction dim = 40
NP = H // 2       # 32 h-pairs
BFREE = 2 * Wt * KS * NT * S * S   # B tile free elems = 2560
BF16 = ml_dtypes.bfloat16

_NC_CACHE = {}


def _build_bass():
    nc = bacc.Bacc()
    # banded masks, one tile per h-pair, matmul-ready:
    #   bmat[HP, g*KB + w~ + dj, hp, w~, di, t, a, b] = m[g,di,dj,2HP+hp,a,t,w~,b]
    bmat_d = nc.declare_dram_parameter(
        "bmat", [NP, KK, BFREE], mybir.dt.bfloat16, isOutput=False)
    # stationary (dense 40 rows, block-diag zeros included):
    #   stat[g*KB + w', r, t, g*64 + cc] = fpad[g*64+cc, r, 16t+w']
    stat_d = nc.declare_dram_parameter(
        "stat", [KK, H, NT, 128], mybir.dt.bfloat16, isOutput=False)
    out_d = nc.declare_dram_parameter(
        "out", [128, S * H, S * W], mybir.dt.bfloat16, isOutput=True)

    NSLOT = 16   # B pair-tile slots (5 KB/partition each)
    HB = 4       # output rows per batched store (= 2 pairs)
    out_rows = out_d.rearrange("c (hb y) x -> c hb (y x)", hb=H // HB)

    # B tile free strides (elems): [hp, w~, di, t, a, b]
    ST_HP = Wt * KS * NT * S * S   # 1280
    ST_W = KS * NT * S * S         # 80
    ST_DI = NT * S * S             # 16
    ST_T = S * S                   # 4

    with tile.TileContext(nc) as tc, ExitStack() as ctx:
        statp = ctx.enter_context(tc.tile_pool(name="statp", bufs=1))
        bp = ctx.enter_context(tc.tile_pool(name="bp", bufs=NSLOT))
        pp = ctx.enter_context(tc.tile_pool(name="pp", bufs=5, space="PSUM"))
        op = ctx.enter_context(tc.tile_pool(name="op", bufs=3))

        btiles = {}
        psums = {}
        otiles = {}
        started = set()

        def load_b(hp):
            bt = bp.tile([KK, 2, Wt, KS, NT, S, S], mybir.dt.bfloat16,
                         name=f"bt{hp}", tag="bt")
            eng = nc.sync if hp % 2 == 0 else nc.scalar
            eng.dma_start(out=bt, in_=bmat_d[hp])
            btiles[hp] = bt

        stats = [None] * (H // 4)

        def load_stat(rb, eng):
            st = statp.tile([KK, 4, NT, 128], mybir.dt.bfloat16,
                            name=f"st{rb}", tag=f"st{rb}")
            eng.dma_start(out=st, in_=stat_d[:, 4 * rb: 4 * rb + 4])
            stats[rb] = st

        load_stat(0, nc.scalar)  # gates MM #1 — first on its queue
        load_b(0)
        load_b(1)
        load_stat(1, nc.sync)
        for hp in range(2, NSLOT):  # warm the remaining B slots
            load_b(hp)

        def first_r(hpair):
            return max(0, 2 * hpair - 2)

        def last_r(hpair):
            return min(H - 1, 2 * hpair + 3)

        for r in range(H):
            # prefetch B pair-tiles ahead of the live window
            for hp in range(max(0, (r - 2) // 2), min(NP - 1, r // 2 + 12) + 1):
                if hp not in btiles:
                    load_b(hp)
            # stat chunk for rows [4rb, 4rb+4) emitted ~10 rows ahead
            rb_need = min(H // 4 - 1, (r + 10) // 4)
            if stats[rb_need] is None:
                load_stat(rb_need, nc.sync if rb_need % 2 else nc.scalar)

            # live pairs at this r
            pairs = []
            for hpair in range((max(0, r - 2)) // 2, min(H - 1, r + 2) // 2 + 1):
                d0 = r + 2 - 2 * hpair   # di of hp=0 row
                pairs.append((hpair, d0))

            for t in range(NT):
                for hpair, d0 in pairs:
                    if hpair not in psums:
                        psums[hpair] = pp.tile(
                            [128, NT, 2, Wt, S, S], mybir.dt.float32,
                            name=f"ps{hpair}", tag="ps")
                    ps = psums[hpair]
                    st = stats[r // 4][:, r % 4, t, :]
                    first = hpair not in started
                    started.add(hpair)
                    stop = (r == last_r(hpair) and t == NT - 1)
                    if d0 == 1:
                        # hp=1's first touch: its PSUM bytes are still
                        # pending-zero while hp=0's are written, and one
                        # matmul may not touch a mixed region (CoreSim
                        # asserts; HW per-element semantics would be fine).
                        # Two singles keep each touch uniform.
                        for hp in range(2):
                            nc.tensor.matmul(
                                out=ps[:, t, hp],
                                lhsT=st,
                                rhs=btiles[hpair][:, hp, :, 1 - hp, t, :, :],
                                start=first, stop=stop and hp == 1,
                                skip_group_check=True)
                            first = False
                    elif 2 <= d0 <= 4:
                        # one MM covers both rows: hp=1 uses di = d0-1 via
                        # an hp step of (hp_stride - di_stride)
                        rhs = bass.AP(
                            tensor=btiles[hpair].tensor,
                            offset=(btiles[hpair].offset
                                    + d0 * ST_DI + t * ST_T),
                            ap=[[BFREE, KK], [ST_HP - ST_DI, 2],
                                [ST_W, Wt], [S, S], [1, S]],
                        )
                        nc.tensor.matmul(
                            out=ps[:, t], lhsT=st, rhs=rhs,
                            start=first, stop=stop, skip_group_check=True)
                    else:
                        hp = 0 if d0 == 0 else 1
                        di = d0 - hp
                        nc.tensor.matmul(
                            out=ps[:, t, hp],
                            lhsT=st,
                            rhs=btiles[hpair][:, hp, :, di, t, :, :],
                            start=first, stop=stop, skip_group_check=True)

            # drain pairs whose last contribution was at r-1
            done = [(r - 4) // 2] if (r >= 4 and r % 2 == 0) else []
            if r == H - 1:
                done += [NP - 2, NP - 1]
            for hpair in done:
                blk = hpair // 2
                if blk not in otiles:
                    otiles[blk] = op.tile([128, HB, S, NT, Wt, S],
                                          mybir.dt.bfloat16,
                                          name=f"ot{blk}", tag="ot")
                for hp in range(2):
                    hl = (hpair % 2) * 2 + hp
                    # ot row layout (a, t, w~, b); psum is (t, hp, w~, a, b)
                    nc.vector.tensor_copy(
                        out=otiles[blk][:, hl],
                        in_=psums[hpair][:, :, hp].rearrange(
                            "c t w a b -> c a t w b"))
                del psums[hpair], btiles[hpair]
                started.discard(hpair)
                if hpair % 2 == 1:
                    eng = nc.sync if blk % 2 == 0 else nc.scalar
                    eng.dma_start(out=out_rows[:, blk], in_=otiles[blk])
                    del otiles[blk]

    nc.finalize()
    return nc


def _host_shards(features, masks):
    """Build per-core stat/bmat arrays (bf16)."""
    in_maps = []
    iw = np.arange(Wt)
    for c in range(8):
        n, j = c // 2, c % 2
        f = features[n, 128 * j: 128 * (j + 1)]        # [128, 64, 64] f32
        m = masks[n, 50 * j: 50 * j + 50]              # [50, 128, 128] f32

        # stationary: stat[g*KB + w', r, t, g*64+cc] = fpad[g*64+cc, r, 16t+w']
        stat = np.zeros((KK, H, NT, 128), np.float32)
        fp = np.pad(f, ((0, 0), (0, 0), (2, 2)))
        for g in range(2):
            for t in range(NT):
                sl = fp[g * 64:(g + 1) * 64, :, Wt * t: Wt * t + KB]
                stat[g * KB:(g + 1) * KB, :, t, g * 64:(g + 1) * 64] = \
                    sl.transpose(2, 1, 0)

        # banded masks per h-pair:
        # B[HP, g*KB + w~ + dj, hp, w~, di, t, a, b] = m[g,di,dj,2HP+hp,a,t,w~,b]
        M8 = m.reshape(2, KS, KS, H, S, NT, Wt, S)     # g,di,dj,h,a,t,w,b
        B = np.zeros((NP, KK, 2, Wt, KS, NT, S, S), np.float32)
        for g in range(2):
            for dj in range(KS):
                src = M8[g, :, dj].reshape(KS, NP, 2, S, NT, Wt, S)
                # dest adv-index dims: [w~, HP, hp, di, t, a, b]
                B[:, g * KB + dj + iw, :, iw] = src.transpose(5, 1, 2, 0, 4, 3, 6)

        in_maps.append({
            "stat": np.ascontiguousarray(stat).astype(BF16),
            "bmat": np.ascontiguousarray(B).reshape(NP, KK, BFREE).astype(BF16),
        })
    return in_maps


def kernel(features, masks, _trace=False):
    features = np.asarray(features, dtype=np.float32)
    masks = np.asarray(masks, dtype=np.float32)

    in_maps = _host_shards(features, masks)

    if "nc" not in _NC_CACHE:
        _NC_CACHE["nc"] = _build_bass()
    nc = _NC_CACHE["nc"]

    res = run_bass_kernel_spmd(nc, in_maps, list(range(8)), trace=_trace)
    kernel._last_result = res

    out = np.empty((N, C, S * H, S * W), np.float32)
    for c in range(8):
        n, j = c // 2, c % 2
        out[n, 128 * j: 128 * (j + 1)] = \
            res.results[c]["out"].astype(np.float32)
    return out


# revision 3
# speedup vs baseline: 1.1396x; 1.1396x over previous
"""CARAFE-naive upsampling (N=4, C=256, H=W=64, k=5, g=4, s=2) on 8 TRN2
NeuronCores.

Sharding: core c <- (batch n = c//2, group-pair j = c%2): 128 feature
channels (2 of the 4 mask groups) of one batch image per core.

Banded-matmul formulation (see v1): per source row r and w-tile of Wt=32
columns, out[(g,c), (w,a,b)] += sum_{w'} statT[(g,w'), (g,c)] * B[(g,w'),
(w,a,b)], where B holds mask values on shifted diagonals and the 5 kernel
rows di accumulate into PSUM over r = h + di - 2.

v3, tuned from v2's trace (v2 = 101.7us, PE 70us busy at 78ns/MM from
un-hidden per-MM LDWEIGHTS; DMA engines 0-9 pinned at 100% for 70us
because 40-partition loads only reach 10 of 16 SDMA engines):
- Wt=32, K=72 exactly: loads span partitions 0-71 -> all 16 engines
  (partition p maps to engine (p//4) mod 16). No K padding: no FWL, but
  no memsets/pad shipping either; LDWEIGHTS (~107ns) hides under the
  larger matmul streams.
- h-QUAD psum packing: 4 adjacent output row-pairs share each (r, t)
  stationary with di differing by 1 -> one matmul with an affine hq-step
  of (hq_stride - di_stride) covers up to 4 rows (N up to 512 = one full
  PSUM bank). ~9 MMs per r instead of v1's 10 small ones, most N>=256.
- B ships as fp8 E3M4 (4 mantissa bits): masks are U[0,1); measured
  end-to-end rel err 1.27% vs the 2% gate (bf16 everything else).
  Mixed-dtype matmul (bf16 stationary x fp8 moving) is HW-supported.
- bytes/core: bmat 5.9 MB + stat 2.36 + out 4.19 = 12.45 MB (v1: 21).
"""

import sys

import numpy as np

for _p in ("/opt/trn_rl_repo", "/opt/pypackages"):
    if _p not in sys.path:
        sys.path.append(_p)

import ml_dtypes  # noqa: E402
from contextlib import ExitStack  # noqa: E402

import concourse.bass as bass  # noqa: E402
import concourse.tile as tile  # noqa: E402
from concourse import bacc, mybir  # noqa: E402
from concourse.bass_utils import run_bass_kernel_spmd  # noqa: E402

KS = 5            # kernel size
S = 2             # upscale
N, C, H, W = 4, 256, 64, 64
Wt = 32           # w-tile
NT = W // Wt      # 2 tiles
KB = Wt + 4       # band rows per group = 36
KK = 2 * KB       # contraction dim = 72
NQ = H // 4       # 16 h-quads
HFREE = Wt * KS * NT * S * S       # per-h free elems = 1280
BFREE = 4 * HFREE                  # quad tile free elems = 5120
BF16 = ml_dtypes.bfloat16
FP8 = ml_dtypes.float8_e3m4

_NC_CACHE = {}


def _build_bass():
    nc = bacc.Bacc()
    # banded masks, one tile per h-quad, fp8 E3M4, matmul-ready:
    #   bmat[Q, g*KB + w + dj, hq, w, di, t, a, b] = m[g,di,dj,4Q+hq,a,t,w,b]
    bmat_d = nc.declare_dram_parameter(
        "bmat", [NQ, KK, BFREE], mybir.dt.float8e3, isOutput=False)
    # stationary (dense 72 rows, block-diag zeros included):
    #   stat[g*KB + w', r, t, g*64 + cc] = fpad[g*64+cc, r, 32t+w']
    stat_d = nc.declare_dram_parameter(
        "stat", [KK, H, NT, 128], mybir.dt.bfloat16, isOutput=False)
    out_d = nc.declare_dram_parameter(
        "out", [128, S * H, S * W], mybir.dt.bfloat16, isOutput=True)

    NSLOT = 8    # B quad-tile slots (5 KB/partition each, fp8)
    HB = 4       # output rows per batched store (= 1 quad)
    out_rows = out_d.rearrange("c (hb y) x -> c hb (y x)", hb=H // HB)

    # B tile free strides (elems): [hq, w, di, t, a, b]
    ST_HQ = HFREE                  # 1280
    ST_W = KS * NT * S * S         # 40
    ST_DI = NT * S * S             # 8
    ST_T = S * S                   # 4

    with tile.TileContext(nc) as tc, ExitStack() as ctx:
        statp = ctx.enter_context(tc.tile_pool(name="statp", bufs=1))
        bp = ctx.enter_context(tc.tile_pool(name="bp", bufs=NSLOT))
        pp = ctx.enter_context(tc.tile_pool(name="pp", bufs=4, space="PSUM"))
        op = ctx.enter_context(tc.tile_pool(name="op", bufs=3))

        btiles = {}
        psums = {}
        started = {}   # quad -> set of started t-banks (bank = t)

        def load_b(q):
            bt = bp.tile([KK, 4, Wt, KS, NT, S, S], mybir.dt.float8e3,
                         name=f"bt{q}", tag="bt")
            eng = nc.sync if q % 2 == 0 else nc.scalar
            eng.dma_start(out=bt, in_=bmat_d[q])
            btiles[q] = bt

        stats = [None] * (H // 4)

        def load_stat(rb, eng):
            st = statp.tile([KK, 4, NT, 128], mybir.dt.bfloat16,
                            name=f"st{rb}", tag=f"st{rb}")
            eng.dma_start(out=st, in_=stat_d[:, 4 * rb: 4 * rb + 4])
            stats[rb] = st

        load_stat(0, nc.scalar)  # gates MM #1 — first on its queue
        load_b(0)
        load_stat(1, nc.sync)
        for q in range(1, NSLOT - 1):  # warm the remaining B slots
            load_b(q)

        def last_r(q):
            return min(H - 1, 4 * q + 5)

        for r in range(H):
            # prefetch B quad-tiles ahead of the live window
            for q in range(max(0, (r - 2) // 4), min(NQ - 1, r // 4 + 5) + 1):
                if q not in btiles:
                    load_b(q)
            rb_need = min(H // 4 - 1, (r + 10) // 4)
            if stats[rb_need] is None:
                load_stat(rb_need, nc.sync if rb_need % 2 else nc.scalar)

            # live quads at this r: rows [r-2, r+2] clipped
            lo, hi = max(0, r - 2), min(H - 1, r + 2)
            for t in range(NT):
                for q in range(lo // 4, hi // 4 + 1):
                    # rows of quad q live at this r, as hq range
                    h0, h1 = max(lo, 4 * q), min(hi, 4 * q + 3)
                    if q not in psums:
                        psums[q] = pp.tile(
                            [128, NT, 4, Wt, S, S], mybir.dt.float32,
                            name=f"ps{q}", tag="ps")
                        started[q] = set()
                    ps = psums[q]
                    st = stats[r // 4][:, r % 4, t, :]
                    bt = btiles[q]
                    first = t not in started[q]
                    started[q].add(t)
                    stop = (r == last_r(q) and t == NT - 1)
                    # rows h0..h1 use di = r+2-h; fresh row (first touch at
                    # this r) is h = r+2 (di=0) — its PSUM bytes are still
                    # pending-zero, so it must be touched by its own MM
                    # (CoreSim requires uniform regions; HW would be fine).
                    nh = h1 - h0 + 1
                    fresh = (h1 == r + 2)
                    if fresh and nh > 1:
                        blocks = [(h0, nh - 1), (h1, 1)]
                    else:
                        blocks = [(h0, nh)]
                    for hb0, cnt in blocks:
                        di0 = r + 2 - hb0   # di of first row in block
                        if cnt == 1:
                            rhs = bt[:, hb0 % 4, :, di0, t, :, :]
                        else:
                            rhs = bass.AP(
                                tensor=bt.tensor,
                                offset=(bt.offset + (hb0 % 4) * ST_HQ
                                        + di0 * ST_DI + t * ST_T),
                                ap=[[BFREE, KK], [ST_HQ - ST_DI, cnt],
                                    [ST_W, Wt], [S, S], [1, S]],
                            )
                        nc.tensor.matmul(
                            out=ps[:, t, hb0 % 4: hb0 % 4 + cnt],
                            lhsT=st, rhs=rhs,
                            start=first, stop=stop and hb0 + cnt - 1 == h1,
                            skip_group_check=True)
                        first = False

            # drain quad q when its last contribution was at r-1
            done = [(r - 6) // 4] if (r >= 6 and (r - 6) % 4 == 0) else []
            if r == H - 1:
                done += [NQ - 1]
            for q in done:
                ot = op.tile([128, HB, S, NT, Wt, S], mybir.dt.bfloat16,
                             name=f"ot{q}", tag="ot")
                for hq in range(4):
                    # ot row layout (a, t, w, b); psum is (t, hq, w, a, b)
                    nc.vector.tensor_copy(
                        out=ot[:, hq],
                        in_=psums[q][:, :, hq].rearrange(
                            "c t w a b -> c a t w b"))
                del psums[q], btiles[q]
                del started[q]
                eng = nc.sync if q % 2 == 0 else nc.scalar
                eng.dma_start(out=out_rows[:, q], in_=ot)

    nc.finalize()
    return nc


def _host_shards(features, masks):
    """Build per-core stat (bf16) / bmat (fp8 e3m4) arrays."""
    in_maps = []
    iw = np.arange(Wt)
    for c in range(8):
        n, j = c // 2, c % 2
        f = features[n, 128 * j: 128 * (j + 1)]        # [128, 64, 64] f32
        m = masks[n, 50 * j: 50 * j + 50]              # [50, 128, 128] f32

        # stat[g*KB + w', r, t, g*64+cc] = fpad[g*64+cc, r, 32t+w']
        stat = np.zeros((KK, H, NT, 128), np.float32)
        fp = np.pad(f, ((0, 0), (0, 0), (2, 2)))
        for g in range(2):
            for t in range(NT):
                sl = fp[g * 64:(g + 1) * 64, :, Wt * t: Wt * t + KB]
                stat[g * KB:(g + 1) * KB, :, t, g * 64:(g + 1) * 64] = \
                    sl.transpose(2, 1, 0)

        # B[Q, g*KB + w + dj, hq, w, di, t, a, b] = m[g,di,dj,4Q+hq,a,t,w,b]
        M8 = m.reshape(2, KS, KS, H, S, NT, Wt, S)     # g,di,dj,h,a,t,w,b
        B = np.zeros((NQ, KK, 4, Wt, KS, NT, S, S), np.float32)
        for g in range(2):
            for dj in range(KS):
                src = M8[g, :, dj].reshape(KS, NQ, 4, S, NT, Wt, S)
                # dest adv-index dims: [w, Q, hq, di, t, a, b]
                B[:, g * KB + dj + iw, :, iw] = src.transpose(5, 1, 2, 0, 4, 3, 6)

        in_maps.append({
            "stat": np.ascontiguousarray(stat).astype(BF16),
            "bmat": np.ascontiguousarray(B).reshape(NQ, KK, BFREE).astype(FP8),
        })
    return in_maps


def kernel(features, masks, _trace=False):
    features = np.asarray(features, dtype=np.float32)
    masks = np.asarray(masks, dtype=np.float32)

    in_maps = _host_shards(features, masks)

    if "nc" not in _NC_CACHE:
        _NC_CACHE["nc"] = _build_bass()
    nc = _NC_CACHE["nc"]

    res = run_bass_kernel_spmd(nc, in_maps, list(range(8)), trace=_trace)
    kernel._last_result = res

    out = np.empty((N, C, S * H, S * W), np.float32)
    for c in range(8):
        n, j = c // 2, c % 2
        out[n, 128 * j: 128 * (j + 1)] = \
            res.results[c]["out"].astype(np.float32)
    return out
